# revision 11
# baseline (speedup 1.0000x reference)
"""CrossModalAttention Trainium2 kernel.

Problem (per batch sample b):
    q = wq @ x_b + bq          (32, 4096)
    k = wk @ ff_b + bk         (32, 4096)
    v = wv @ ff_b + bv         (256, 4096)
    s[i, j] = sum_d q[d, i] k[d, j]
    p = softmax_j(s)
    out[c, i] = sum_j v[c, j] p[i, j]
    y = gamma * (wo @ out + bo) + x_b

Sharding: data-parallel over batch, one sample per NeuronCore (B == 8 cores).

Per-core algorithm (everything stays on-chip after the initial loads):
  - weights are pre-transposed on host so every conv is a plain matmul
    with contraction on partitions; all matmul operands are bf16 (1
    cycle/row on the PE; fp32/f32r are 4x/1.5x slower), accumulation is
    fp32 in PSUM.
  - scores are computed TRANSPOSED (sT[j, i]) so the PV contraction
    (over j) has j on partitions for both operands; softmax-over-j then
    needs a cross-partition sum, done by an extra all-ones stationary
    matmul accumulated alongside PV (output rows are the denominators
    replicated across all 128 partitions - no broadcast needed for the
    subsequent normalize).
  - exp() without max-subtraction (scores are O(1); mathematically
    identical to softmax with max-subtraction).
  - v-bias and the output bias are folded on host into the residual:
    xg = x + gamma*(wo @ bv + bo), so the epilogue is one vector op
    fin = gamma*(wo @ onorm) + xg. With gamma == 0 the output is
    bit-exact x.
"""

import numpy as np

try:
    import ml_dtypes

    _BF16 = ml_dtypes.bfloat16
except Exception:  # pragma: no cover
    _BF16 = None

import concourse.bass as bass
import concourse.tile as tile
from concourse import bacc, mybir
from concourse.bass_utils import run_bass_kernel_spmd

B, C, HH, WW = 8, 256, 64, 64
CR = 32
N = HH * WW  # 4096
P = 128
CT = C // P  # 2 c-tiles
IT = 512  # i-tile width
NI = N // IT  # 8
NJ = N // P  # 32 j-tiles
N_CORES = 8

F32 = mybir.dt.float32
BF16 = mybir.dt.bfloat16


def build_nc():
    nc = bacc.Bacc("TRN2", target_bir_lowering=False, debug=False, num_devices=N_CORES)

    xg_d = nc.dram_tensor("xg", (CT, P, N), F32, kind="ExternalInput")
    xb_d = nc.dram_tensor("xb", (CT, P, N), BF16, kind="ExternalInput")
    ffb_d = nc.dram_tensor("ffb", (CT, P, N), BF16, kind="ExternalInput")
    wqT_d = nc.dram_tensor("wqT", (CT, P, CR), BF16, kind="ExternalInput")
    wkT_d = nc.dram_tensor("wkT", (CT, P, CR), BF16, kind="ExternalInput")
    wvT_d = nc.dram_tensor("wvT", (CT, P, C), BF16, kind="ExternalInput")
    woT_d = nc.dram_tensor("woT", (CT, P, C), BF16, kind="ExternalInput")
    bq_d = nc.dram_tensor("bq", (CR, 1), F32, kind="ExternalInput")
    bk_d = nc.dram_tensor("bk", (CR, 1), F32, kind="ExternalInput")
    gamma_d = nc.dram_tensor("gamma_bc", (P, 1), F32, kind="ExternalInput")
    ones_d = nc.dram_tensor("ones", (P, P), BF16, kind="ExternalInput")
    out_d = nc.dram_tensor("out", (CT, P, N), F32, kind="ExternalOutput")

    with tile.TileContext(nc) as tc:
        with (
            tc.tile_pool(name="singles", bufs=1) as singles,
            tc.tile_pool(name="eT", bufs=5) as p_eT,
            tc.tile_pool(name="recip", bufs=2) as p_recip,
            tc.tile_pool(name="onorm", bufs=2) as p_onorm,
            tc.tile_pool(name="final", bufs=4) as p_final,
            tc.tile_pool(name="ps_mm", bufs=4, space="PSUM") as ps_mm,
            tc.tile_pool(name="ps_out", bufs=3, space="PSUM") as ps_out,
            tc.tile_pool(name="ps_den", bufs=1, space="PSUM") as ps_den,
        ):
            # ---- persistent SBUF tensors ----
            xg_sb = singles.tile([P, CT, N], F32)
            xb_sb = singles.tile([P, CT, N], BF16)
            ffb_sb = singles.tile([P, CT, N], BF16)
            q_sb = singles.tile([CR, N], BF16)
            k_sb = singles.tile([CR, N], BF16)
            vT_sb = singles.tile([P, NJ, C], BF16)
            wqT_sb = singles.tile([P, CT, CR], BF16)
            wkT_sb = singles.tile([P, CT, CR], BF16)
            wvT_sb = singles.tile([P, CT, C], BF16)
            woT_sb = singles.tile([P, CT, C], BF16)
            bq_sb = singles.tile([CR, 1], F32)
            bk_sb = singles.tile([CR, 1], F32)
            gamma_sb = singles.tile([P, 1], F32)
            ones_sb = singles.tile([P, P], BF16)

            # ---- loads ----
            nc.sync.dma_start(out=wqT_sb, in_=wqT_d.rearrange("t p m -> p t m"))
            nc.sync.dma_start(out=wkT_sb, in_=wkT_d.rearrange("t p m -> p t m"))
            nc.sync.dma_start(out=wvT_sb, in_=wvT_d.rearrange("t p m -> p t m"))
            nc.sync.dma_start(out=woT_sb, in_=woT_d.rearrange("t p m -> p t m"))
            nc.sync.dma_start(out=bq_sb, in_=bq_d[:, :])
            nc.sync.dma_start(out=bk_sb, in_=bk_d[:, :])
            nc.sync.dma_start(out=gamma_sb, in_=gamma_d[:, :])
            nc.sync.dma_start(out=ones_sb, in_=ones_d[:, :])
            # bulk activations, chunked for DMA-queue parallelism
            NCH = 4
            CW = N // NCH
            for t in range(CT):
                for ch in range(NCH):
                    sl = bass.ds(ch * CW, CW)
                    nc.sync.dma_start(out=ffb_sb[:, t, sl], in_=ffb_d[t, :, sl])
            for t in range(CT):
                for ch in range(NCH):
                    sl = bass.ds(ch * CW, CW)
                    nc.sync.dma_start(out=xb_sb[:, t, sl], in_=xb_d[t, :, sl])
            for t in range(CT):
                for ch in range(NCH):
                    sl = bass.ds(ch * CW, CW)
                    nc.sync.dma_start(out=xg_sb[:, t, sl], in_=xg_d[t, :, sl])

            # ---- q = wq @ x + bq ; k = wk @ ff + bk  (rows 0..31) ----
            for src_sb, wT_sb, b_sb, dst_sb in (
                (ffb_sb, wkT_sb, bk_sb, k_sb),
                (xb_sb, wqT_sb, bq_sb, q_sb),
            ):
                for i in range(NI):
                    isl = bass.ds(i * IT, IT)
                    qp = ps_mm.tile([CR, IT], F32, tag="mm", name="qp")
                    for t in range(CT):
                        nc.tensor.matmul(
                            qp,
                            wT_sb[:, t, :],
                            src_sb[:, t, isl],
                            start=(t == 0),
                            stop=(t == CT - 1),
                        )
                    nc.scalar.activation(
                        dst_sb[0:CR, isl],
                        qp,
                        mybir.ActivationFunctionType.Identity,
                        bias=b_sb,
                    )

            # ---- vT[j, c] = sum_c' ff[c', j] * wvT[c', c]  (no bias) ----
            for jt in range(NJ):
                jsl = bass.ds(jt * P, P)
                vp = ps_mm.tile([P, C], F32, tag="mm", name="vp")
                for t in range(CT):
                    nc.tensor.matmul(
                        vp,
                        ffb_sb[:, t, jsl],
                        wvT_sb[:, t, :],
                        start=(t == 0),
                        stop=(t == CT - 1),
                    )
                nc.vector.tensor_copy(vT_sb[:, jt, :], vp)

            # ---- attention main loop over i-tiles ----
            for i in range(NI):
                isl = bass.ds(i * IT, IT)
                outU = [
                    ps_out.tile([P, IT], F32, tag="outU", name=f"outU{t}")
                    for t in range(CT)
                ]
                den = ps_den.tile([P, IT], F32, tag="den")

                # software pipeline: emit sT/exp two j-tiles ahead of the PV
                # group so the PE never stalls on the sT->exp->PV chain.
                def emit_score_exp(jt, isl=isl):
                    with tc.high_priority(offset=20):
                        jsl = bass.ds(jt * P, P)
                        sT = ps_mm.tile([P, IT], F32, tag="mm", name="sT")
                        nc.tensor.matmul(
                            sT, k_sb[:, jsl], q_sb[:, isl], start=True, stop=True
                        )
                        eT = p_eT.tile([P, IT], BF16, name="eT")
                        nc.scalar.activation(eT, sT, mybir.ActivationFunctionType.Exp)
                    return eT

                LOOKAHEAD = 3
                eT_queue = [emit_score_exp(jt) for jt in range(LOOKAHEAD)]
                for jt in range(NJ):
                    if jt + LOOKAHEAD < NJ:
                        eT_queue.append(emit_score_exp(jt + LOOKAHEAD))
                    eT = eT_queue.pop(0)
                    first, last = jt == 0, jt == NJ - 1
                    for t in range(CT):
                        nc.tensor.matmul(
                            outU[t],
                            vT_sb[:, jt, t * P : (t + 1) * P],
                            eT,
                            start=first,
                            stop=last,
                        )
                    nc.tensor.matmul(den, ones_sb, eT, start=first, stop=last)

                recip = p_recip.tile([P, IT], F32)
                nc.vector.reciprocal_approx_fast(recip, den)
                onorm = p_onorm.tile([P, CT, IT], BF16)
                for t in range(CT):
                    nc.vector.tensor_mul(onorm[:, t, :], outU[t], recip)

                # wo conv + epilogue: y = gamma*(wo@onorm) + (x + gamma*(wo@bv+bo))
                for ct in range(CT):
                    wf = ps_mm.tile([P, IT], F32, tag="mm", name="wf")
                    for t in range(CT):
                        nc.tensor.matmul(
                            wf,
                            woT_sb[:, t, ct * P : (ct + 1) * P],
                            onorm[:, t, :],
                            start=(t == 0),
                            stop=(t == CT - 1),
                        )
                    fin = p_final.tile([P, IT], F32)
                    nc.vector.scalar_tensor_tensor(
                        out=fin,
                        in0=wf,
                        scalar=gamma_sb,
                        in1=xg_sb[:, ct, isl],
                        op0=mybir.AluOpType.mult,
                        op1=mybir.AluOpType.add,
                    )
                    nc.sync.dma_start(out=out_d[ct, :, isl], in_=fin)

    nc.compile()
    return nc


_NC_CACHE = None


def _get_nc():
    global _NC_CACHE
    if _NC_CACHE is None:
        _NC_CACHE = build_nc()
    return _NC_CACHE


_ONES = np.ones((P, P), dtype=np.float32)


def _bf16(a):
    return np.ascontiguousarray(a.astype(_BF16))


def _make_in_maps(inputs):
    x = np.asarray(inputs["x"], dtype=np.float32)
    ff = np.asarray(inputs["cross_modal_features"], dtype=np.float32)
    wq = np.asarray(inputs["wq"], dtype=np.float32)
    bq = np.asarray(inputs["bq"], dtype=np.float32)
    wk = np.asarray(inputs["wk"], dtype=np.float32)
    bk = np.asarray(inputs["bk"], dtype=np.float32)
    wv = np.asarray(inputs["wv"], dtype=np.float32)
    bv = np.asarray(inputs["bv"], dtype=np.float32)
    wo = np.asarray(inputs["wo"], dtype=np.float32)
    bo = np.asarray(inputs["bo"], dtype=np.float32)
    gamma = np.asarray(inputs["gamma"], dtype=np.float32)

    g = float(gamma[0])
    wqT = _bf16(np.ascontiguousarray(wq.T).reshape(CT, P, CR))
    wkT = _bf16(np.ascontiguousarray(wk.T).reshape(CT, P, CR))
    wvT = _bf16(np.ascontiguousarray(wv.T).reshape(CT, P, C))
    woT = _bf16(np.ascontiguousarray(wo.T).reshape(CT, P, C))
    gamma_bc = np.full((P, 1), g, dtype=np.float32)
    gbo = (g * (wo.astype(np.float64) @ bv.astype(np.float64) + bo)).astype(np.float32)
    ones_bf = _bf16(_ONES)
    bq_c = bq.reshape(CR, 1).copy()
    bk_c = bk.reshape(CR, 1).copy()

    in_maps = []
    for b in range(B):
        xb_full = x[b].reshape(CT, P, N)
        xg = (xb_full + gbo.reshape(CT, P, 1)).astype(np.float32)
        in_maps.append(
            {
                "xg": np.ascontiguousarray(xg),
                "xb": _bf16(xb_full),
                "ffb": _bf16(ff[b].reshape(CT, P, N)),
                "wqT": wqT,
                "wkT": wkT,
                "wvT": wvT,
                "woT": woT,
                "bq": bq_c,
                "bk": bk_c,
                "gamma_bc": gamma_bc,
                "ones": ones_bf,
            }
        )
    return in_maps


def kernel(x, cross_modal_features, wq, bq, wk, bk, wv, bv, wo, bo, gamma):
    inputs = {
        "x": x,
        "cross_modal_features": cross_modal_features,
        "wq": wq,
        "bq": bq,
        "wk": wk,
        "bk": bk,
        "wv": wv,
        "bv": bv,
        "wo": wo,
        "bo": bo,
        "gamma": gamma,
    }
    nc = _get_nc()
    in_maps = _make_in_maps(inputs)
    res = run_bass_kernel_spmd(nc, in_maps, core_ids=list(range(N_CORES)))

    out = np.empty((B, C, HH, WW), dtype=np.float32)
    for b in range(B):
        out[b] = res.results[b]["out"].reshape(C, HH, WW)
    return out


if __name__ == "__main__":
    rng = np.random.default_rng(0)
    ins = {
        "x": rng.standard_normal((B, C, HH, WW), dtype=np.float32),
        "cross_modal_features": rng.standard_normal((B, C, HH, WW), dtype=np.float32),
        "wq": (rng.standard_normal((CR, C), dtype=np.float32) * 0.02),
        "bq": np.zeros(CR, np.float32),
        "wk": (rng.standard_normal((CR, C), dtype=np.float32) * 0.02),
        "bk": np.zeros(CR, np.float32),
        "wv": (rng.standard_normal((C, C), dtype=np.float32) * 0.02),
        "bv": np.zeros(C, np.float32),
        "wo": (rng.standard_normal((C, C), dtype=np.float32) * 0.02),
        "bo": np.zeros(C, np.float32),
        "gamma": np.ones(1, np.float32),
    }
    y = kernel(**ins)
    print("out", y.shape, y.dtype, float(np.abs(y).max()))


# revision 12
# speedup vs baseline: 1.1141x; 1.1141x over previous
"""CrossModalAttention Trainium2 kernel.

Problem (per batch sample b):
    q = wq @ x_b + bq          (32, 4096)
    k = wk @ ff_b + bk         (32, 4096)
    v = wv @ ff_b + bv         (256, 4096)
    s[i, j] = sum_d q[d, i] k[d, j]
    p = softmax_j(s)
    out[c, i] = sum_j v[c, j] p[i, j]
    y = gamma * (wo @ out + bo) + x_b

Sharding: data-parallel over batch, one sample per NeuronCore (B == 8 cores).

Per-core algorithm (everything stays on-chip after the initial loads):
  - weights are pre-transposed on host so every conv is a plain matmul
    with contraction on partitions; all matmul operands are bf16 (1
    cycle/row on the PE; fp32/f32r are 4x/1.5x slower), accumulation is
    fp32 in PSUM.
  - scores are computed TRANSPOSED (sT[j, i]) so the PV contraction
    (over j) has j on partitions for both operands; softmax-over-j then
    needs a cross-partition sum, done by an extra all-ones stationary
    matmul accumulated alongside PV (output rows are the denominators
    replicated across all 128 partitions - no broadcast needed for the
    subsequent normalize).
  - exp() without max-subtraction (scores are O(1); mathematically
    identical to softmax with max-subtraction).
  - v-bias and the output bias are folded on host into the residual:
    xg = x + gamma*(wo @ bv + bo), so the epilogue is one vector op
    fin = gamma*(wo @ onorm) + xg. With gamma == 0 the output is
    bit-exact x.
"""

import numpy as np

try:
    import ml_dtypes

    _BF16 = ml_dtypes.bfloat16
except Exception:  # pragma: no cover
    _BF16 = None

import concourse.bass as bass
import concourse.tile as tile
from concourse import bacc, mybir
from concourse.bass_utils import run_bass_kernel_spmd

B, C, HH, WW = 8, 256, 64, 64
CR = 32
N = HH * WW  # 4096
P = 128
CT = C // P  # 2 c-tiles
IT = 512  # i-tile width
NI = N // IT  # 8
NJ = N // P  # 32 j-tiles
N_CORES = 8

F32 = mybir.dt.float32
BF16 = mybir.dt.bfloat16


def build_nc():
    nc = bacc.Bacc("TRN2", target_bir_lowering=False, debug=False, num_devices=N_CORES)

    xg_d = nc.dram_tensor("xg", (CT, P, N), F32, kind="ExternalInput")
    xb_d = nc.dram_tensor("xb", (CT, P, N), BF16, kind="ExternalInput")
    ffb_d = nc.dram_tensor("ffb", (CT, P, N), BF16, kind="ExternalInput")
    wqT_d = nc.dram_tensor("wqT", (CT, P, CR), BF16, kind="ExternalInput")
    wkT_d = nc.dram_tensor("wkT", (CT, P, CR), BF16, kind="ExternalInput")
    wvT_d = nc.dram_tensor("wvT", (CT, P, C), BF16, kind="ExternalInput")
    woT_d = nc.dram_tensor("woT", (CT, P, C), BF16, kind="ExternalInput")
    bq_d = nc.dram_tensor("bq", (CR, 1), F32, kind="ExternalInput")
    bk_d = nc.dram_tensor("bk", (CR, 1), F32, kind="ExternalInput")
    gamma_d = nc.dram_tensor("gamma_bc", (P, 1), F32, kind="ExternalInput")
    ones_d = nc.dram_tensor("ones", (P, P), BF16, kind="ExternalInput")
    out_d = nc.dram_tensor("out", (CT, P, N), F32, kind="ExternalOutput")

    with tile.TileContext(nc) as tc:
        with (
            tc.tile_pool(name="singles", bufs=1) as singles,
            tc.tile_pool(name="eT", bufs=2) as p_eT,
            tc.tile_pool(name="recip", bufs=2) as p_recip,
            tc.tile_pool(name="onorm", bufs=2) as p_onorm,
            tc.tile_pool(name="final", bufs=4) as p_final,
            tc.tile_pool(name="ps_mm", bufs=1, space="PSUM") as ps_mm,
            tc.tile_pool(name="ps_out", bufs=1, space="PSUM") as ps_out,
            tc.tile_pool(name="ps_den", bufs=1, space="PSUM") as ps_den,
        ):
            # ---- persistent SBUF tensors ----
            xg_sb = singles.tile([P, CT, N], F32)
            xb_sb = singles.tile([P, CT, N], BF16)
            ffb_sb = singles.tile([P, CT, N], BF16)
            q_sb = singles.tile([CR, N], BF16)
            k_sb = singles.tile([CR, N], BF16)
            vT_sb = singles.tile([P, NJ, C], BF16)
            wqT_sb = singles.tile([P, CT, CR], BF16)
            wkT_sb = singles.tile([P, CT, CR], BF16)
            wvT_sb = singles.tile([P, CT, C], BF16)
            woT_sb = singles.tile([P, CT, C], BF16)
            bq_sb = singles.tile([CR, 1], F32)
            bk_sb = singles.tile([CR, 1], F32)
            gamma_sb = singles.tile([P, 1], F32)
            ones_sb = singles.tile([P, P], BF16)

            # ---- loads ----
            nc.sync.dma_start(out=wqT_sb, in_=wqT_d.rearrange("t p m -> p t m"))
            nc.sync.dma_start(out=wkT_sb, in_=wkT_d.rearrange("t p m -> p t m"))
            nc.sync.dma_start(out=wvT_sb, in_=wvT_d.rearrange("t p m -> p t m"))
            nc.sync.dma_start(out=woT_sb, in_=woT_d.rearrange("t p m -> p t m"))
            nc.sync.dma_start(out=bq_sb, in_=bq_d[:, :])
            nc.sync.dma_start(out=bk_sb, in_=bk_d[:, :])
            nc.sync.dma_start(out=gamma_sb, in_=gamma_d[:, :])
            nc.sync.dma_start(out=ones_sb, in_=ones_d[:, :])
            # bulk activations, chunked for DMA-queue parallelism
            NCH = 4
            CW = N // NCH
            for t in range(CT):
                for ch in range(NCH):
                    sl = bass.ds(ch * CW, CW)
                    nc.sync.dma_start(out=ffb_sb[:, t, sl], in_=ffb_d[t, :, sl])
            for t in range(CT):
                for ch in range(NCH):
                    sl = bass.ds(ch * CW, CW)
                    nc.sync.dma_start(out=xb_sb[:, t, sl], in_=xb_d[t, :, sl])
            for t in range(CT):
                for ch in range(NCH):
                    sl = bass.ds(ch * CW, CW)
                    nc.sync.dma_start(out=xg_sb[:, t, sl], in_=xg_d[t, :, sl])

            # ---- q = wq @ x + bq ; k = wk @ ff + bk  (rows 0..31) ----
            for src_sb, wT_sb, b_sb, dst_sb in (
                (ffb_sb, wkT_sb, bk_sb, k_sb),
                (xb_sb, wqT_sb, bq_sb, q_sb),
            ):
                for i in range(NI):
                    isl = bass.ds(i * IT, IT)
                    qp = ps_mm.tile([CR, IT], F32, tag=f"sT{i % 4}", name="qp")
                    for t in range(CT):
                        nc.tensor.matmul(
                            qp,
                            wT_sb[:, t, :],
                            src_sb[:, t, isl],
                            start=(t == 0),
                            stop=(t == CT - 1),
                        )
                    nc.scalar.activation(
                        dst_sb[0:CR, isl],
                        qp,
                        mybir.ActivationFunctionType.Identity,
                        bias=b_sb,
                    )

            # ---- vT[j, c] = sum_c' ff[c', j] * wvT[c', c]  (no bias) ----
            for jt in range(NJ):
                jsl = bass.ds(jt * P, P)
                vp = ps_mm.tile([P, C], F32, tag=f"sT{jt % 4}", name="vp")
                for t in range(CT):
                    nc.tensor.matmul(
                        vp,
                        ffb_sb[:, t, jsl],
                        wvT_sb[:, t, :],
                        start=(t == 0),
                        stop=(t == CT - 1),
                    )
                nc.vector.tensor_copy(vT_sb[:, jt, :], vp)

            # ---- attention main loop over i-tiles ----
            for i in range(NI):
                isl = bass.ds(i * IT, IT)
                outU = [
                    ps_out.tile([P, IT], F32, tag=f"outU{t}", name=f"outU{t}")
                    for t in range(CT)
                ]
                den = ps_den.tile([P, IT], F32, tag="den")

                # software pipeline: emit sT/exp two j-tiles ahead of the PV
                # group so the PE never stalls on the sT->exp->PV chain.
                def emit_score_exp(jt, isl=isl):
                    jsl = bass.ds(jt * P, P)
                    sT = ps_mm.tile([P, IT], F32, tag=f"sT{jt % 4}", name="sT")
                    nc.tensor.matmul(
                        sT, k_sb[:, jsl], q_sb[:, isl], start=True, stop=True
                    )
                    eT = p_eT.tile([P, IT], BF16, tag=f"eT{jt % 4}", name="eT")
                    nc.scalar.activation(eT, sT, mybir.ActivationFunctionType.Exp)
                    return eT

                LOOKAHEAD = 3
                eT_queue = [emit_score_exp(jt) for jt in range(LOOKAHEAD)]
                for jt in range(NJ):
                    if jt + LOOKAHEAD < NJ:
                        eT_queue.append(emit_score_exp(jt + LOOKAHEAD))
                    eT = eT_queue.pop(0)
                    first, last = jt == 0, jt == NJ - 1
                    for t in range(CT):
                        nc.tensor.matmul(
                            outU[t],
                            vT_sb[:, jt, t * P : (t + 1) * P],
                            eT,
                            start=first,
                            stop=last,
                        )
                    nc.tensor.matmul(den, ones_sb, eT, start=first, stop=last)

                recip = p_recip.tile([P, IT], F32)
                nc.vector.reciprocal_approx_fast(recip, den)
                onorm = p_onorm.tile([P, CT, IT], BF16)
                for t in range(CT):
                    nc.vector.tensor_mul(onorm[:, t, :], outU[t], recip)

                # wo conv + epilogue: y = gamma*(wo@onorm) + (x + gamma*(wo@bv+bo))
                for ct in range(CT):
                    wf = ps_mm.tile([P, IT], F32, tag="mm", name="wf")
                    for t in range(CT):
                        nc.tensor.matmul(
                            wf,
                            woT_sb[:, t, ct * P : (ct + 1) * P],
                            onorm[:, t, :],
                            start=(t == 0),
                            stop=(t == CT - 1),
                        )
                    fin = p_final.tile([P, IT], F32)
                    nc.vector.scalar_tensor_tensor(
                        out=fin,
                        in0=wf,
                        scalar=gamma_sb,
                        in1=xg_sb[:, ct, isl],
                        op0=mybir.AluOpType.mult,
                        op1=mybir.AluOpType.add,
                    )
                    nc.sync.dma_start(out=out_d[ct, :, isl], in_=fin)

    nc.compile()
    return nc


_NC_CACHE = None


def _get_nc():
    global _NC_CACHE
    if _NC_CACHE is None:
        _NC_CACHE = build_nc()
    return _NC_CACHE


_ONES = np.ones((P, P), dtype=np.float32)


def _bf16(a):
    return np.ascontiguousarray(a.astype(_BF16))


def _make_in_maps(inputs):
    x = np.asarray(inputs["x"], dtype=np.float32)
    ff = np.asarray(inputs["cross_modal_features"], dtype=np.float32)
    wq = np.asarray(inputs["wq"], dtype=np.float32)
    bq = np.asarray(inputs["bq"], dtype=np.float32)
    wk = np.asarray(inputs["wk"], dtype=np.float32)
    bk = np.asarray(inputs["bk"], dtype=np.float32)
    wv = np.asarray(inputs["wv"], dtype=np.float32)
    bv = np.asarray(inputs["bv"], dtype=np.float32)
    wo = np.asarray(inputs["wo"], dtype=np.float32)
    bo = np.asarray(inputs["bo"], dtype=np.float32)
    gamma = np.asarray(inputs["gamma"], dtype=np.float32)

    g = float(gamma[0])
    wqT = _bf16(np.ascontiguousarray(wq.T).reshape(CT, P, CR))
    wkT = _bf16(np.ascontiguousarray(wk.T).reshape(CT, P, CR))
    wvT = _bf16(np.ascontiguousarray(wv.T).reshape(CT, P, C))
    woT = _bf16(np.ascontiguousarray(wo.T).reshape(CT, P, C))
    gamma_bc = np.full((P, 1), g, dtype=np.float32)
    gbo = (g * (wo.astype(np.float64) @ bv.astype(np.float64) + bo)).astype(np.float32)
    ones_bf = _bf16(_ONES)
    bq_c = bq.reshape(CR, 1).copy()
    bk_c = bk.reshape(CR, 1).copy()

    in_maps = []
    for b in range(B):
        xb_full = x[b].reshape(CT, P, N)
        xg = (xb_full + gbo.reshape(CT, P, 1)).astype(np.float32)
        in_maps.append(
            {
                "xg": np.ascontiguousarray(xg),
                "xb": _bf16(xb_full),
                "ffb": _bf16(ff[b].reshape(CT, P, N)),
                "wqT": wqT,
                "wkT": wkT,
                "wvT": wvT,
                "woT": woT,
                "bq": bq_c,
                "bk": bk_c,
                "gamma_bc": gamma_bc,
                "ones": ones_bf,
            }
        )
    return in_maps


def kernel(x, cross_modal_features, wq, bq, wk, bk, wv, bv, wo, bo, gamma):
    inputs = {
        "x": x,
        "cross_modal_features": cross_modal_features,
        "wq": wq,
        "bq": bq,
        "wk": wk,
        "bk": bk,
        "wv": wv,
        "bv": bv,
        "wo": wo,
        "bo": bo,
        "gamma": gamma,
    }
    nc = _get_nc()
    in_maps = _make_in_maps(inputs)
    res = run_bass_kernel_spmd(nc, in_maps, core_ids=list(range(N_CORES)))

    out = np.empty((B, C, HH, WW), dtype=np.float32)
    for b in range(B):
        out[b] = res.results[b]["out"].reshape(C, HH, WW)
    return out


if __name__ == "__main__":
    rng = np.random.default_rng(0)
    ins = {
        "x": rng.standard_normal((B, C, HH, WW), dtype=np.float32),
        "cross_modal_features": rng.standard_normal((B, C, HH, WW), dtype=np.float32),
        "wq": (rng.standard_normal((CR, C), dtype=np.float32) * 0.02),
        "bq": np.zeros(CR, np.float32),
        "wk": (rng.standard_normal((CR, C), dtype=np.float32) * 0.02),
        "bk": np.zeros(CR, np.float32),
        "wv": (rng.standard_normal((C, C), dtype=np.float32) * 0.02),
        "bv": np.zeros(C, np.float32),
        "wo": (rng.standard_normal((C, C), dtype=np.float32) * 0.02),
        "bo": np.zeros(C, np.float32),
        "gamma": np.ones(1, np.float32),
    }
    y = kernel(**ins)
    print("out", y.shape, y.dtype, float(np.abs(y).max()))


# revision 13
# speedup vs baseline: 1.1161x; 1.0018x over previous
"""CrossModalAttention Trainium2 kernel.

Problem (per batch sample b):
    q = wq @ x_b + bq          (32, 4096)
    k = wk @ ff_b + bk         (32, 4096)
    v = wv @ ff_b + bv         (256, 4096)
    s[i, j] = sum_d q[d, i] k[d, j]
    p = softmax_j(s)
    out[c, i] = sum_j v[c, j] p[i, j]
    y = gamma * (wo @ out + bo) + x_b

Sharding: data-parallel over batch, one sample per NeuronCore (B == 8 cores).

Per-core algorithm (everything stays on-chip after the initial loads):
  - weights are pre-transposed on host so every conv is a plain matmul
    with contraction on partitions; all matmul operands are bf16 (1
    cycle/row on the PE; fp32/f32r are 4x/1.5x slower), accumulation is
    fp32 in PSUM.
  - scores are computed TRANSPOSED (sT[j, i]) so the PV contraction
    (over j) has j on partitions for both operands; softmax-over-j then
    needs a cross-partition sum, done by an extra all-ones stationary
    matmul accumulated alongside PV (output rows are the denominators
    replicated across all 128 partitions - no broadcast needed for the
    subsequent normalize).
  - exp() without max-subtraction (scores are O(1); mathematically
    identical to softmax with max-subtraction).
  - v-bias and the output bias are folded on host into the residual:
    xg = x + gamma*(wo @ bv + bo), so the epilogue is one vector op
    fin = gamma*(wo @ onorm) + xg. With gamma == 0 the output is
    bit-exact x.
"""

import numpy as np

try:
    import ml_dtypes

    _BF16 = ml_dtypes.bfloat16
except Exception:  # pragma: no cover
    _BF16 = None

import concourse.bass as bass
import concourse.tile as tile
from concourse import bacc, mybir
from concourse.bass_utils import run_bass_kernel_spmd

B, C, HH, WW = 8, 256, 64, 64
CR = 32
N = HH * WW  # 4096
P = 128
CT = C // P  # 2 c-tiles
IT = 512  # i-tile width
NI = N // IT  # 8
NJ = N // P  # 32 j-tiles
N_CORES = 8

F32 = mybir.dt.float32
BF16 = mybir.dt.bfloat16


def build_nc():
    nc = bacc.Bacc("TRN2", target_bir_lowering=False, debug=False, num_devices=N_CORES)

    xg_d = nc.dram_tensor("xg", (CT, P, N), F32, kind="ExternalInput")
    xb_d = nc.dram_tensor("xb", (CT, P, N), BF16, kind="ExternalInput")
    ffb_d = nc.dram_tensor("ffb", (CT, P, N), BF16, kind="ExternalInput")
    wqT_d = nc.dram_tensor("wqT", (CT, P, CR), BF16, kind="ExternalInput")
    wkT_d = nc.dram_tensor("wkT", (CT, P, CR), BF16, kind="ExternalInput")
    wvT_d = nc.dram_tensor("wvT", (CT, P, C), BF16, kind="ExternalInput")
    woT_d = nc.dram_tensor("woT", (CT, P, C), BF16, kind="ExternalInput")
    bq_d = nc.dram_tensor("bq", (CR, 1), F32, kind="ExternalInput")
    bk_d = nc.dram_tensor("bk", (CR, 1), F32, kind="ExternalInput")
    gamma_d = nc.dram_tensor("gamma_bc", (P, 1), F32, kind="ExternalInput")
    ones_d = nc.dram_tensor("ones", (P, P), BF16, kind="ExternalInput")
    out_d = nc.dram_tensor("out", (CT, P, N), F32, kind="ExternalOutput")

    with tile.TileContext(nc) as tc:
        with (
            tc.tile_pool(name="singles", bufs=1) as singles,
            tc.tile_pool(name="eT", bufs=2) as p_eT,
            tc.tile_pool(name="recip", bufs=2) as p_recip,
            tc.tile_pool(name="onorm", bufs=2) as p_onorm,
            tc.tile_pool(name="final", bufs=4) as p_final,
            tc.tile_pool(name="ps_mm", bufs=1, space="PSUM") as ps_mm,
            tc.tile_pool(name="ps_out", bufs=1, space="PSUM") as ps_out,
            tc.tile_pool(name="ps_den", bufs=1, space="PSUM") as ps_den,
        ):
            # ---- persistent SBUF tensors ----
            xg_sb = singles.tile([P, CT, N], F32)
            xb_sb = singles.tile([P, CT, N], BF16)
            ffb_sb = singles.tile([P, CT, N], BF16)
            q_sb = singles.tile([CR, N], BF16)
            k_sb = singles.tile([CR, N], BF16)
            vT_sb = singles.tile([P, NJ, C], BF16)
            wqT_sb = singles.tile([P, CT, CR], BF16)
            wkT_sb = singles.tile([P, CT, CR], BF16)
            wvT_sb = singles.tile([P, CT, C], BF16)
            woT_sb = singles.tile([P, CT, C], BF16)
            bq_sb = singles.tile([CR, 1], F32)
            bk_sb = singles.tile([CR, 1], F32)
            gamma_sb = singles.tile([P, 1], F32)
            ones_sb = singles.tile([P, P], BF16)

            # ---- loads ----
            nc.sync.dma_start(out=wqT_sb, in_=wqT_d.rearrange("t p m -> p t m"))
            nc.sync.dma_start(out=wkT_sb, in_=wkT_d.rearrange("t p m -> p t m"))
            nc.sync.dma_start(out=wvT_sb, in_=wvT_d.rearrange("t p m -> p t m"))
            nc.sync.dma_start(out=woT_sb, in_=woT_d.rearrange("t p m -> p t m"))
            nc.sync.dma_start(out=bq_sb, in_=bq_d[:, :])
            nc.sync.dma_start(out=bk_sb, in_=bk_d[:, :])
            nc.sync.dma_start(out=gamma_sb, in_=gamma_d[:, :])
            nc.sync.dma_start(out=ones_sb, in_=ones_d[:, :])
            # bulk activations, chunked for DMA-queue parallelism
            NCH = 4
            CW = N // NCH
            engs = [nc.sync, nc.gpsimd]
            for t in range(CT):
                for ch in range(NCH):
                    sl = bass.ds(ch * CW, CW)
                    engs[ch % 2].dma_start(out=ffb_sb[:, t, sl], in_=ffb_d[t, :, sl])
            for t in range(CT):
                for ch in range(NCH):
                    sl = bass.ds(ch * CW, CW)
                    engs[ch % 2].dma_start(out=xb_sb[:, t, sl], in_=xb_d[t, :, sl])
            for t in range(CT):
                for ch in range(NCH):
                    sl = bass.ds(ch * CW, CW)
                    engs[ch % 2].dma_start(out=xg_sb[:, t, sl], in_=xg_d[t, :, sl])

            # ---- q = wq @ x + bq ; k = wk @ ff + bk  (rows 0..31) ----
            for src_sb, wT_sb, b_sb, dst_sb in (
                (ffb_sb, wkT_sb, bk_sb, k_sb),
                (xb_sb, wqT_sb, bq_sb, q_sb),
            ):
                for i in range(NI):
                    isl = bass.ds(i * IT, IT)
                    qp = ps_mm.tile([CR, IT], F32, tag=f"sT{i % 4}", name="qp")
                    for t in range(CT):
                        nc.tensor.matmul(
                            qp,
                            wT_sb[:, t, :],
                            src_sb[:, t, isl],
                            start=(t == 0),
                            stop=(t == CT - 1),
                        )
                    nc.scalar.activation(
                        dst_sb[0:CR, isl],
                        qp,
                        mybir.ActivationFunctionType.Identity,
                        bias=b_sb,
                    )

            # ---- vT[j, c] = sum_c' ff[c', j] * wvT[c', c]  (no bias) ----
            for jt in range(NJ):
                jsl = bass.ds(jt * P, P)
                vp = ps_mm.tile([P, C], F32, tag=f"sT{jt % 4}", name="vp")
                for t in range(CT):
                    nc.tensor.matmul(
                        vp,
                        ffb_sb[:, t, jsl],
                        wvT_sb[:, t, :],
                        start=(t == 0),
                        stop=(t == CT - 1),
                    )
                nc.vector.tensor_copy(vT_sb[:, jt, :], vp)

            # ---- attention main loop over i-tiles ----
            for i in range(NI):
                isl = bass.ds(i * IT, IT)
                outU = [
                    ps_out.tile([P, IT], F32, tag=f"outU{t}", name=f"outU{t}")
                    for t in range(CT)
                ]
                den = ps_den.tile([P, IT], F32, tag="den")

                # software pipeline: emit sT/exp two j-tiles ahead of the PV
                # group so the PE never stalls on the sT->exp->PV chain.
                def emit_score_exp(jt, isl=isl):
                    with tc.high_priority(offset=20):
                        jsl = bass.ds(jt * P, P)
                        sT = ps_mm.tile([P, IT], F32, tag=f"sT{jt % 4}", name="sT")
                        nc.tensor.matmul(
                            sT, k_sb[:, jsl], q_sb[:, isl], start=True, stop=True
                        )
                        eT = p_eT.tile([P, IT], BF16, tag=f"eT{jt % 4}", name="eT")
                        nc.scalar.activation(eT, sT, mybir.ActivationFunctionType.Exp)
                    return eT

                LOOKAHEAD = 3
                eT_queue = [emit_score_exp(jt) for jt in range(LOOKAHEAD)]
                for jt in range(NJ):
                    if jt + LOOKAHEAD < NJ:
                        eT_queue.append(emit_score_exp(jt + LOOKAHEAD))
                    eT = eT_queue.pop(0)
                    first, last = jt == 0, jt == NJ - 1
                    for t in range(CT):
                        nc.tensor.matmul(
                            outU[t],
                            vT_sb[:, jt, t * P : (t + 1) * P],
                            eT,
                            start=first,
                            stop=last,
                        )
                    nc.tensor.matmul(den, ones_sb, eT, start=first, stop=last)

                recip = p_recip.tile([P, IT], F32)
                nc.vector.reciprocal_approx_fast(recip, den)
                onorm = p_onorm.tile([P, CT, IT], BF16)
                for t in range(CT):
                    nc.vector.tensor_mul(onorm[:, t, :], outU[t], recip)

                # wo conv + epilogue: y = gamma*(wo@onorm) + (x + gamma*(wo@bv+bo))
                for ct in range(CT):
                    wf = ps_mm.tile([P, IT], F32, tag="mm", name="wf")
                    for t in range(CT):
                        nc.tensor.matmul(
                            wf,
                            woT_sb[:, t, ct * P : (ct + 1) * P],
                            onorm[:, t, :],
                            start=(t == 0),
                            stop=(t == CT - 1),
                        )
                    fin = p_final.tile([P, IT], F32)
                    nc.vector.scalar_tensor_tensor(
                        out=fin,
                        in0=wf,
                        scalar=gamma_sb,
                        in1=xg_sb[:, ct, isl],
                        op0=mybir.AluOpType.mult,
                        op1=mybir.AluOpType.add,
                    )
                    nc.sync.dma_start(out=out_d[ct, :, isl], in_=fin)

    nc.compile()
    return nc


_NC_CACHE = None


def _get_nc():
    global _NC_CACHE
    if _NC_CACHE is None:
        _NC_CACHE = build_nc()
    return _NC_CACHE


_ONES = np.ones((P, P), dtype=np.float32)


def _bf16(a):
    return np.ascontiguousarray(a.astype(_BF16))


def _make_in_maps(inputs):
    x = np.asarray(inputs["x"], dtype=np.float32)
    ff = np.asarray(inputs["cross_modal_features"], dtype=np.float32)
    wq = np.asarray(inputs["wq"], dtype=np.float32)
    bq = np.asarray(inputs["bq"], dtype=np.float32)
    wk = np.asarray(inputs["wk"], dtype=np.float32)
    bk = np.asarray(inputs["bk"], dtype=np.float32)
    wv = np.asarray(inputs["wv"], dtype=np.float32)
    bv = np.asarray(inputs["bv"], dtype=np.float32)
    wo = np.asarray(inputs["wo"], dtype=np.float32)
    bo = np.asarray(inputs["bo"], dtype=np.float32)
    gamma = np.asarray(inputs["gamma"], dtype=np.float32)

    g = float(gamma[0])
    wqT = _bf16(np.ascontiguousarray(wq.T).reshape(CT, P, CR))
    wkT = _bf16(np.ascontiguousarray(wk.T).reshape(CT, P, CR))
    wvT = _bf16(np.ascontiguousarray(wv.T).reshape(CT, P, C))
    woT = _bf16(np.ascontiguousarray(wo.T).reshape(CT, P, C))
    gamma_bc = np.full((P, 1), g, dtype=np.float32)
    gbo = (g * (wo.astype(np.float64) @ bv.astype(np.float64) + bo)).astype(np.float32)
    ones_bf = _bf16(_ONES)
    bq_c = bq.reshape(CR, 1).copy()
    bk_c = bk.reshape(CR, 1).copy()

    in_maps = []
    for b in range(B):
        xb_full = x[b].reshape(CT, P, N)
        xg = (xb_full + gbo.reshape(CT, P, 1)).astype(np.float32)
        in_maps.append(
            {
                "xg": np.ascontiguousarray(xg),
                "xb": _bf16(xb_full),
                "ffb": _bf16(ff[b].reshape(CT, P, N)),
                "wqT": wqT,
                "wkT": wkT,
                "wvT": wvT,
                "woT": woT,
                "bq": bq_c,
                "bk": bk_c,
                "gamma_bc": gamma_bc,
                "ones": ones_bf,
            }
        )
    return in_maps


def kernel(x, cross_modal_features, wq, bq, wk, bk, wv, bv, wo, bo, gamma):
    inputs = {
        "x": x,
        "cross_modal_features": cross_modal_features,
        "wq": wq,
        "bq": bq,
        "wk": wk,
        "bk": bk,
        "wv": wv,
        "bv": bv,
        "wo": wo,
        "bo": bo,
        "gamma": gamma,
    }
    nc = _get_nc()
    in_maps = _make_in_maps(inputs)
    res = run_bass_kernel_spmd(nc, in_maps, core_ids=list(range(N_CORES)))

    out = np.empty((B, C, HH, WW), dtype=np.float32)
    for b in range(B):
        out[b] = res.results[b]["out"].reshape(C, HH, WW)
    return out


if __name__ == "__main__":
    rng = np.random.default_rng(0)
    ins = {
        "x": rng.standard_normal((B, C, HH, WW), dtype=np.float32),
        "cross_modal_features": rng.standard_normal((B, C, HH, WW), dtype=np.float32),
        "wq": (rng.standard_normal((CR, C), dtype=np.float32) * 0.02),
        "bq": np.zeros(CR, np.float32),
        "wk": (rng.standard_normal((CR, C), dtype=np.float32) * 0.02),
        "bk": np.zeros(CR, np.float32),
        "wv": (rng.standard_normal((C, C), dtype=np.float32) * 0.02),
        "bv": np.zeros(C, np.float32),
        "wo": (rng.standard_normal((C, C), dtype=np.float32) * 0.02),
        "bo": np.zeros(C, np.float32),
        "gamma": np.ones(1, np.float32),
    }
    y = kernel(**ins)
    print("out", y.shape, y.dtype, float(np.abs(y).max()))
